# revision 25
# baseline (speedup 1.0000x reference)
"""Trainium2 Bass kernel for nn_MinJerkReg (min-jerk quadratic cost + trajectory
regularizer loss).

Math
----
reference() = quad + rho * reg where
  quad = sum_{p,i,j} C[p,i] cost_mat[i,j] C[p,j],   C = coeff[:4] reshaped (4,1024)
  reg  = w_reg[:14] @ x0 + sum_{n,s} w_reg[14+14n+s] * ref[s,n]
  ref[s,n] = degree-<=7 polynomial of the segment-local time dt_n with
             coefficients derived from coeff.

Device decomposition (8 cores, 16 of the 128 segments each, ~125k steps/core):
  Per segment, timesteps are laid out (128 partitions x 62 steps); with the
  shift identity dt(u,q) = dtb_u + q*h the traj value at (u, q, s) is
  sum_e pow[u,e] * G'[seg, e, (q,s)].  Contract the w stream against pow on
  the TENSOR engine (w is the matmul moving operand):
      WP[seg, e, qs] = sum_u pow[u, e] * w[u, seg, qs]
  so reg_core = sum_{seg,e,qs} WP[seg,e,qs] * G'[seg,e,qs].  Each segment's
  WP is an (8 x 868) PSUM tile; using a zero-padded 32-wide stationary
  (4 segments per PE column quadrant, accumulated with start/stop groups)
  the 16 segments pack exactly into PSUM partitions p = 8*seg + e of two
  banks.  Per 4-segment chunk, one DVE multiply against the matching G'
  stack and one ACT accum-reduce fold the chunk's 32 PSUM rows into an
  accumulator column; all but the last chunk's finish hides under the DMA
  stream.

  The 128x62 tiling (not 123x64) is deliberate: SWDGE descriptors map to
  the 16 SDMA engines by destination SBUF partition, so a 128-partition w
  tensor is the only shape that spreads the 1.7 MB stream across every
  engine (123 partitions leaves 4 engines idle).  w is quantized host-side
  to fp8e4 (x256 scale) and pow/G' to fp8 as well (x1 / x0.25) -- all the
  quantization noise is random-sign into a 14M-term dot, ~1e-4 relative
  effect vs the 2e-2 gate.  w streams via gpsimd SWDGE in [2,4,4,4,2]-seg
  chunks (small first chunk for an early PE start, small last chunk so
  only ~1us of matmul trails the final DMA); the tiny operands ride the
  SP/Activation HWDGE queues during the SWDGE ramp so the SWDGE ring
  carries nothing but w.  Measured per-core DMA fabric ceiling with all 8
  cores streaming is ~250-280 GB/s no matter how transfers are split
  across rings, so the whole kernel is stream-bound: exec ~= fixed boot
  (~3.6us) + 1.86 MB stream (~7us) + trailing matmul+reduce (~4us) +
  fixed runtime epilogue (~5us).
  quad: cost_mat is verified to equal kron(eye(128), Q8) (it is by
  construction); quad = <Q8, sum_b C_b^T C_b> needs one K=64 f32r matmul +
  a tiny DVE/ACT finish (host falls back to an exact f64 einsum if the
  structure check ever fails).  Host sums per-core accumulator columns in
  float64 and applies x0/rho.

This toolchain permits exactly ONE semaphore wait per instruction, so the
kernel is raw Bass (no Tile): extra dependencies are standalone wait_ge
instructions with hand-counted semaphore arithmetic.
"""

import numpy as np

import concourse.bass as bass
import concourse.mybir as mybir
from concourse.bass_utils import run_bass_kernel_spmd

F32 = mybir.dt.float32
F32R = mybir.dt.float32r
BF16 = mybir.dt.bfloat16
F8 = mybir.dt.float8e4
W_SCALE = 256.0
GP_SCALE = 0.25                        # gp stored as fp8 * GP_SCALE
AOT = mybir.AluOpType

N_CORES = 8
NUM_SEG = 128
SEG_PER_CORE = NUM_SEG // N_CORES     # 16
ORDER = 7
NC8 = ORDER + 1                        # 8 polynomial coefficients / powers
M_STEPS = 62                           # timesteps per partition
NPART = 128                            # partitions per segment tile (127 active)
FREE = 14 * M_STEPS                    # 868 floats per partition per segment
HA = 512                               # first matmul free chunk (full PSUM bank)
HB = FREE - HA                         # 356 (exposed tail chunk is the small one)
N_WCHUNK = 5                           # w DMA chunks (seg ranges below)

# module global: last BassKernelResults (for test harness introspection)
LAST_RESULTS = None


def _falling(j, d):
    return float(np.prod(np.arange(j, j - d, -1))) if j >= d else 0.0


def _build_nc():
    nc = bass.Bass(trn_type="TRN2", num_devices=N_CORES, debug=False)
    wq = nc.dram_tensor("wq", [NPART, SEG_PER_CORE * FREE], F8, kind="ExternalInput").ap()
    pw = nc.dram_tensor("pw", [128, SEG_PER_CORE * 32], F8, kind="ExternalInput").ap()
    gp = nc.dram_tensor("gp", [128, FREE], F8, kind="ExternalInput").ap()
    ckq = nc.dram_tensor("ckq", [64, 16], F32R, kind="ExternalInput").ap()
    acc_out = nc.dram_tensor("acc_out", [128, 4], F32, kind="ExternalOutput").ap()

    import contextlib
    ctx = contextlib.ExitStack()
    with ctx:
        wqt = ctx.enter_context(nc.sbuf_tensor([NPART, SEG_PER_CORE * FREE], F8))
        pwt = ctx.enter_context(nc.sbuf_tensor([128, SEG_PER_CORE * 32], F8))
        gpt = ctx.enter_context(nc.sbuf_tensor([128, FREE], F8))
        ckqt = ctx.enter_context(nc.sbuf_tensor([64, 16], F32R))
        scrap = ctx.enter_context(nc.sbuf_tensor([128, FREE], F32))
        scrap2 = ctx.enter_context(nc.sbuf_tensor([128, FREE], F32))
        scrapq = ctx.enter_context(nc.sbuf_tensor([8, 8], F32))
        scrapq2 = ctx.enter_context(nc.sbuf_tensor([8, 8], F32))
        acc = ctx.enter_context(nc.sbuf_tensor([128, 4], F32))
        # 2 banks: WP halves live at free offsets 0 and 512 (434 used each)
        ps = ctx.enter_context(nc.psum_tensor("ps", [128, 1024], F32))
        psq = ctx.enter_context(nc.psum_tensor("psq", [128, 512], F32))
        psd = ctx.enter_context(nc.psum_tensor("psd", [128, 512], F32))

        s_sy = ctx.enter_context(nc.semaphore(name="s_sy"))     # ckq
        s_pw = ctx.enter_context(nc.semaphore(name="s_pw"))
        s_gp = ctx.enter_context(nc.semaphore(name="s_gp"))
        s_w = [ctx.enter_context(nc.semaphore(name=f"s_w{c}")) for c in range(N_WCHUNK)]
        s_pe = ctx.enter_context(nc.semaphore(name="s_pe"))
        s_dve = ctx.enter_context(nc.semaphore(name="s_dve"))
        s_act = ctx.enter_context(nc.semaphore(name="s_act"))

        block = ctx.enter_context(nc.Block(no_gpsimd_drain=True))

        # DMA plan: the whole w stream rides the gpsimd SWDGE ring (the only
        # ring that sustains ~280 GB/s) in chunks of [2,4,4,4,2] segments —
        # small first chunk so the PE starts early, small last chunk so only
        # ~0.7us of matmul remains after the final DMA lands.  The tiny fp8
        # pw/gp/ckq operands ride the SP/Activation HWDGE rings and arrive
        # during the SWDGE ramp-up.
        SWCH = [(0, 2), (2, 6), (6, 10), (10, 14), (14, 16)]

        @block.gpsimd
        def _(gpsimd):
            for c, (lo, hi) in enumerate(SWCH):
                gpsimd.dma_start(
                    wqt.ap()[:, lo * FREE:hi * FREE],
                    wq[:, lo * FREE:hi * FREE],
                ).then_inc(s_w[c], 16)

        @block.sync
        def _(sync):
            sync.dma_start(pwt.ap(), pw).then_inc(s_pw, 16)
            sync.dma_start(ckqt.ap(), ckq).then_inc(s_sy, 16)
            sync.wait_ge(s_act, 3)
            sync.dma_start(acc_out, acc.ap()).then_inc(s_sy, 16)

        @block.tensor
        def _(tensor):
            # warm up the PE p-state ramp on throwaway matmuls while w lands
            tensor.wait_ge(s_pw, 16)
            for _i in range(3):
                tensor.matmul(
                    psd.ap()[0:32, 0:HA],
                    pwt.ap()[:NPART, 0:32],
                    pwt.ap()[:NPART, 0:HA],
                    start=True, stop=True,
                )
            # quad: only needs ckq, runs while w still streams in
            tensor.wait_ge(s_sy, 16)
            tensor.matmul(
                psq.ap()[:8, 0:8],
                ckqt.ap()[:, 0:8],
                ckqt.ap()[:, 0:8],
                start=True, stop=True,
            ).then_inc(s_pe, 1)
            # 16 segments in 4 PSUM quadrants; within a quadrant run all four
            # h=0 halves then all four h=1 halves so the h=0 accumulation
            # groups close a few matmuls before the very end (the DVE h=0
            # multiply overlaps the tail h=1 matmuls of the last quadrant).
            # chunk needed before (g, j) in the h=0 pass: seg s = 4g+j maps
            # to chunk 0:{0,1} 1:{2..5} 2:{6..9} 3:{10..13} 4:{14,15}; wait
            # right before the first matmul that consumes each chunk.
            CHUNK_WAIT = {(0, 0): 0, (0, 2): 1, (1, 2): 2, (2, 2): 3, (3, 2): 4}
            for g in range(4):
                for h in range(2):
                    for j in range(4):
                        s = 4 * g + j
                        if h == 0 and (g, j) in ((0, 2), (1, 2)):
                            # keep the PE p-state ramp alive across the two
                            # chunk waits that usually find the PE idle
                            for _i in range(2 if g == 0 else 1):
                                tensor.matmul(
                                    psd.ap()[0:32, 0:HA],
                                    pwt.ap()[:NPART, 0:32],
                                    pwt.ap()[:NPART, 0:HA],
                                    start=True, stop=True,
                                )
                        if h == 0 and (g, j) in CHUNK_WAIT:
                            tensor.wait_ge(s_w[CHUNK_WAIT[(g, j)]], 16)
                        lo, hi = (0, HA) if h == 0 else (HA, FREE)
                        mm = tensor.matmul(
                            ps.ap()[32 * g:32 * g + 32, lo:hi],
                            pwt.ap()[:NPART, 32 * s:32 * s + 32],
                            wqt.ap()[:NPART, s * FREE + lo:s * FREE + hi],
                            start=(j == 0), stop=(j == 3),
                            tile_position=(0, 32 * g),
                        )
                        if g == 3 and j == 3:
                            mm.then_inc(s_pe, 1)

        @block.vector
        def _(vector):
            vector.memset(acc.ap(), 0.0)
            # tiny quad product: psq[:8,:8] * q8
            vector.wait_ge(s_pe, 1)
            vector.tensor_mul(
                out=scrapq.ap(),
                in0=psq.ap()[:8, 0:8],
                in1=ckqt.ap()[:8, 8:16].bitcast(F32),
            ).then_inc(s_dve, 1)
            # WP * G' in two free-halves so the ACT reduce pipelines behind
            vector.wait_ge(s_gp, 16)
            vector.wait_ge(s_pe, 2)
            vector.tensor_mul(
                out=scrap.ap()[:, 0:HA],
                in0=ps.ap()[:, 0:HA],
                in1=gpt.ap()[:, 0:HA],
            ).then_inc(s_dve, 1)
            vector.wait_ge(s_pe, 3)
            vector.tensor_mul(
                out=scrap.ap()[:, HA:FREE],
                in0=ps.ap()[:, HA:FREE],
                in1=gpt.ap()[:, HA:FREE],
            ).then_inc(s_dve, 1)

        @block.scalar
        def _(scalar):
            scalar.dma_start(gpt.ap(), gp).then_inc(s_gp, 16)
            scalar.wait_ge(s_dve, 1)
            scalar.activation(
                out=scrapq2.ap(), in_=scrapq.ap(),
                func=mybir.ActivationFunctionType.Copy,
                accum_out=acc.ap()[:8, 1:2],
            ).then_inc(s_act, 1)
            scalar.wait_ge(s_dve, 2)
            scalar.activation(
                out=scrap2.ap()[:, 0:HA], in_=scrap.ap()[:, 0:HA],
                func=mybir.ActivationFunctionType.Copy,
                accum_out=acc.ap()[:, 0:1],
            ).then_inc(s_act, 1)
            scalar.wait_ge(s_dve, 3)
            scalar.activation(
                out=scrap2.ap()[:, HA:FREE], in_=scrap.ap()[:, HA:FREE],
                func=mybir.ActivationFunctionType.Copy,
                accum_out=acc.ap()[:, 2:3],
            ).then_inc(s_act, 1)

    return nc


def _precompute(coeff, cost_mat, ts, w, num_steps):
    """Host-side prep of the tiny per-core operands + padded w blocks."""
    N = int(num_steps)
    ts = np.asarray(ts, np.float32)
    coeff = np.asarray(coeff, np.float32)
    w = np.asarray(w, np.float32)

    times = np.linspace(np.float32(ts[0]), np.float32(ts[-1]), N, dtype=np.float32)
    k = np.searchsorted(ts[1:-1], times, side="left")
    counts = np.bincount(k, minlength=NUM_SEG)
    starts = np.concatenate([[0], np.cumsum(counts)[:-1]]).astype(np.int64)
    assert counts.max() <= NPART * M_STEPS

    # G[seg, s, e] : per-output-row polynomial coefficients in dt^e
    d_of_s = np.array([0, 0, 0, 1, 1, 1, 2, 2, 2, 3, 3, 3, 0, 1])
    a_of_s = np.array([0, 1, 2, 0, 1, 2, 0, 1, 2, 0, 1, 2, 3, 3])
    G = np.zeros((NUM_SEG, 14, NC8), np.float64)
    for s in range(14):
        d, a = int(d_of_s[s]), int(a_of_s[s])
        for e in range(NC8 - d):
            G[:, s, e] = _falling(e + d, d) * coeff[a, :, e + d].astype(np.float64)

    # T[q, e, e'] = C(e,e') (q h)^(e-e')
    from math import comb
    h = (np.float64(ts[-1]) - np.float64(ts[0])) / (N - 1)
    T = np.zeros((M_STEPS, NC8, NC8), np.float64)
    for q in range(M_STEPS):
        for e in range(NC8):
            for ep in range(e + 1):
                T[q, e, ep] = comb(e, ep) * (q * h) ** (e - ep)
    Gp = np.einsum("qef,kse->kqsf", T, G)              # (128, 62, 14, 8)
    rhs_all = np.ascontiguousarray(
        Gp.transpose(0, 3, 1, 2).reshape(NUM_SEG, NC8, FREE)).astype(np.float32)

    # per-partition base dt powers (zeros for inactive partitions)
    u = np.arange(NPART)
    n_act = -(-counts // M_STEPS)                      # ceil
    idx = np.minimum(starts[:, None] + M_STEPS * u[None, :], N - 1)
    dtb = times[idx].astype(np.float64) - ts.astype(np.float64)[:NUM_SEG, None]
    mask = u[None, :] < n_act[:, None]
    dtb = dtb * mask
    pows = dtb[:, None, :] ** np.arange(NC8)[None, :, None]   # (128, 8, 128)
    pows = pows * mask[:, None, :]                            # [seg, e, u]

    # padded per-segment w blocks, scaled and quantized to fp8 e4m3
    f8np = mybir.dt.np(F8)
    w_scaled = (w[14:].astype(np.float32) * np.float32(W_SCALE)).astype(f8np)
    wb_all = np.zeros((NUM_SEG, NPART * FREE), f8np)
    for seg in range(NUM_SEG):
        st, cnt = int(starts[seg]), int(counts[seg])
        wb_all[seg, : 14 * cnt] = w_scaled[14 * st: 14 * (st + cnt)]
    wb_all = wb_all.reshape(NUM_SEG, NPART, FREE)

    cost_mat = np.asarray(cost_mat, np.float32)
    q8b = np.ascontiguousarray(cost_mat[:NC8, :NC8])

    bf16 = mybir.dt.np(BF16)
    in_maps = []
    for c in range(N_CORES):
        sl = slice(c * SEG_PER_CORE, (c + 1) * SEG_PER_CORE)
        wbc = wb_all[sl]                                  # (16, 128, 868)
        wbc = wbc.transpose(1, 0, 2).reshape(NPART, SEG_PER_CORE * FREE)

        # zero-padded 32-wide stationaries: pw[u, 32 s + 8 (s%4) + e] = pow
        pw = np.zeros((128, SEG_PER_CORE * 32), np.float32)
        pc = pows[sl]                                     # (16, 8, 128)
        for s in range(SEG_PER_CORE):
            base = 32 * s + 8 * (s % 4)
            pw[:NPART, base:base + 8] = pc[s].T

        # G' stack matching the packed PSUM layout: gp[8 s + e, :] = G'[seg s, e]
        gpc = rhs_all[sl].reshape(SEG_PER_CORE * NC8, FREE)

        ckq = np.zeros((64, 16), np.float32)
        ckq[:, 0:8] = coeff[:4, sl, :].reshape(4 * SEG_PER_CORE, NC8)
        ckq[0:8, 8:16] = q8b

        f8lim = np.float32(448.0)
        pw8 = np.clip(pw, -f8lim, f8lim).astype(f8np)
        gp8 = np.clip(gpc * np.float32(GP_SCALE), -f8lim, f8lim).astype(f8np)
        in_maps.append({
            "wq": np.ascontiguousarray(wbc),
            "pw": np.ascontiguousarray(pw8),
            "gp": np.ascontiguousarray(gp8),
            "ckq": np.ascontiguousarray(ckq),
        })
    return in_maps


def _install_ntff_hook_shim():
    """The agent image lacks ``antenv.axon_hooks``; recreate it so
    run_bass_kernel_spmd's trace=True path can find the NTFF profile hook
    (test-only; the grading path never passes _trace)."""
    import sys, types
    if "antenv.axon_hooks" in sys.modules:
        return
    import antenv
    mod = types.ModuleType("antenv.axon_hooks")
    _h = [None]
    mod.set_axon_ntff_profile_hook = lambda h: _h.__setitem__(0, h)
    mod.get_axon_ntff_profile_hook = lambda: _h[0]
    sys.modules["antenv.axon_hooks"] = mod
    antenv.axon_hooks = mod
    try:
        from trn_agent_boot.trn_boot import _ntff_profile_via_ctypes
        mod.set_axon_ntff_profile_hook(
            _ntff_profile_via_ctypes("/opt/axon/libaxon_pjrt.so"))
    except Exception as e:
        print("ntff hook shim failed:", e)


def kernel(coeff, cost_mat, ts, x0, w_reg, rho, p, num_steps,
           _trace=False, _trace_cores=None):
    global LAST_RESULTS
    coeff = np.asarray(coeff)
    cost_mat = np.asarray(cost_mat)
    ts = np.asarray(ts)
    x0 = np.asarray(x0)
    w_reg = np.asarray(w_reg)
    assert int(p) == 4 and int(num_steps) == 1_000_000

    cost_mat32 = np.asarray(cost_mat, np.float32)
    q8b = cost_mat32[:NC8, :NC8]
    kron_ok = np.array_equal(
        cost_mat32, np.kron(np.eye(NUM_SEG, dtype=np.float32), q8b))
    in_maps = _precompute(coeff, cost_mat, ts, w_reg, num_steps)
    nc = _build_nc()
    kwargs = {}
    if _trace:
        _install_ntff_hook_shim()
        kwargs = dict(trace=True, trace_cores=_trace_cores or [0])
    res = run_bass_kernel_spmd(nc, in_maps, list(range(N_CORES)), **kwargs)
    LAST_RESULTS = res

    quad = 0.0
    reg = 0.0
    for c in range(N_CORES):
        acc = np.asarray(res.results[c]["acc_out"], np.float64)
        reg += (acc[:, 0] + acc[:, 2]).sum() / (W_SCALE * GP_SCALE)
        quad += acc[:8, 1].sum()
    reg += float(np.asarray(w_reg[:14], np.float64) @ np.asarray(x0, np.float64))
    if not kron_ok:
        # cost_mat without the expected kron structure: the on-device quad
        # fast path does not apply; recompute the (tiny) quadratic exactly.
        C = np.asarray(coeff, np.float64)[:4].reshape(4, -1)
        quad = float(np.einsum("pi,ij,pj->", C, np.asarray(cost_mat, np.float64), C))
    return np.float32(quad + float(rho) * reg)


# revision 26
# speedup vs baseline: 1.0587x; 1.0587x over previous
"""Trainium2 Bass kernel for nn_MinJerkReg (min-jerk quadratic cost + trajectory
regularizer loss).

Math
----
reference() = quad + rho * reg where
  quad = sum_{p,i,j} C[p,i] cost_mat[i,j] C[p,j],   C = coeff[:4] reshaped (4,1024)
  reg  = w_reg[:14] @ x0 + sum_{n,s} w_reg[14+14n+s] * ref[s,n]
  ref[s,n] = degree-<=7 polynomial of the segment-local time dt_n with
             coefficients derived from coeff.

Device decomposition (8 cores, 16 of the 128 segments each, ~125k steps/core):
  Per segment, timesteps are laid out (128 partitions x 62 steps); with the
  shift identity dt(u,q) = dtb_u + q*h the traj value at (u, q, s) is
  sum_e pow[u,e] * G'[seg, e, (q,s)].  Contract the w stream against pow on
  the TENSOR engine (w is the matmul moving operand):
      WP[seg, e, qs] = sum_u pow[u, e] * w[u, seg, qs]
  so reg_core = sum_{seg,e,qs} WP[seg,e,qs] * G'[seg,e,qs].  Each segment's
  WP is an (8 x 868) PSUM tile; using a zero-padded 32-wide stationary
  (4 segments per PE column quadrant, accumulated with start/stop groups)
  the 16 segments pack exactly into PSUM partitions p = 8*seg + e of two
  banks.  Per 4-segment chunk, one DVE multiply against the matching G'
  stack and one ACT accum-reduce fold the chunk's 32 PSUM rows into an
  accumulator column; all but the last chunk's finish hides under the DMA
  stream.

  The 128x62 tiling (not 123x64) is deliberate: SWDGE descriptors map to
  the 16 SDMA engines by destination SBUF partition, so a 128-partition w
  tensor is the only shape that spreads the 1.7 MB stream across every
  engine (123 partitions leaves 4 engines idle).  w is quantized host-side
  to fp8e4 (x256 scale) and pow/G' to fp8 as well (x1 / x0.25) -- all the
  quantization noise is random-sign into a 14M-term dot, ~1e-4 relative
  effect vs the 2e-2 gate.  w streams via gpsimd SWDGE in [2,4,4,4,2]-seg
  chunks (small first chunk for an early PE start, small last chunk so
  only ~1us of matmul trails the final DMA); the tiny operands ride the
  SP/Activation HWDGE queues during the SWDGE ramp so the SWDGE ring
  carries nothing but w.  PSUM halves are split 512/356 (not 434/434) so
  the exposed second-half multiply/reduce after the last matmul is the
  small one; dummy matmuls parked before the two chunk waits that find
  the PE idle keep the p-state ramp alive through DMA jitter.  Measured per-core DMA fabric ceiling with all 8
  cores streaming is ~250-280 GB/s no matter how transfers are split
  across rings, so the whole kernel is stream-bound: exec ~= fixed boot
  (~3.6us) + 1.86 MB stream (~7us) + trailing matmul+reduce (~4us) +
  fixed runtime epilogue (~5us).
  quad: cost_mat is verified to equal kron(eye(128), Q8) (it is by
  construction); quad = <Q8, sum_b C_b^T C_b> needs one K=64 f32r matmul +
  a tiny DVE/ACT finish (host falls back to an exact f64 einsum if the
  structure check ever fails).  Host sums per-core accumulator columns in
  float64 and applies x0/rho.

This toolchain permits exactly ONE semaphore wait per instruction, so the
kernel is raw Bass (no Tile): extra dependencies are standalone wait_ge
instructions with hand-counted semaphore arithmetic.
"""

import numpy as np

import concourse.bass as bass
import concourse.mybir as mybir
from concourse.bass_utils import run_bass_kernel_spmd

F32 = mybir.dt.float32
F32R = mybir.dt.float32r
BF16 = mybir.dt.bfloat16
F8 = mybir.dt.float8e4
W_SCALE = 256.0
GP_SCALE = 0.25                        # gp stored as fp8 * GP_SCALE
AOT = mybir.AluOpType

N_CORES = 8
NUM_SEG = 128
SEG_PER_CORE = NUM_SEG // N_CORES     # 16
ORDER = 7
NC8 = ORDER + 1                        # 8 polynomial coefficients / powers
M_STEPS = 62                           # timesteps per partition
NPART = 128                            # partitions per segment tile (127 active)
FREE = 14 * M_STEPS                    # 868 floats per partition per segment
HA = 512                               # first matmul free chunk (full PSUM bank)
HB = FREE - HA                         # 356 (exposed tail chunk is the small one)
N_WCHUNK = 5                           # w DMA chunks (seg ranges below)

# module global: last BassKernelResults (for test harness introspection)
LAST_RESULTS = None


def _falling(j, d):
    return float(np.prod(np.arange(j, j - d, -1))) if j >= d else 0.0


def _build_nc():
    nc = bass.Bass(trn_type="TRN2", num_devices=N_CORES, debug=False)
    wq = nc.dram_tensor("wq", [NPART, SEG_PER_CORE * FREE], F8, kind="ExternalInput").ap()
    pw = nc.dram_tensor("pw", [128, SEG_PER_CORE * 32], F8, kind="ExternalInput").ap()
    gp = nc.dram_tensor("gp", [128, FREE], F8, kind="ExternalInput").ap()
    ckq = nc.dram_tensor("ckq", [64, 16], F32R, kind="ExternalInput").ap()
    acc_out = nc.dram_tensor("acc_out", [128, 4], F32, kind="ExternalOutput").ap()

    import contextlib
    ctx = contextlib.ExitStack()
    with ctx:
        wqt = ctx.enter_context(nc.sbuf_tensor([NPART, SEG_PER_CORE * FREE], F8))
        pwt = ctx.enter_context(nc.sbuf_tensor([128, SEG_PER_CORE * 32], F8))
        gpt = ctx.enter_context(nc.sbuf_tensor([128, FREE], F8))
        ckqt = ctx.enter_context(nc.sbuf_tensor([64, 16], F32R))
        scrap = ctx.enter_context(nc.sbuf_tensor([128, FREE], F32))
        scrap2 = ctx.enter_context(nc.sbuf_tensor([128, FREE], F32))
        scrapq = ctx.enter_context(nc.sbuf_tensor([8, 8], F32))
        scrapq2 = ctx.enter_context(nc.sbuf_tensor([8, 8], F32))
        acc = ctx.enter_context(nc.sbuf_tensor([128, 4], F32))
        # 2 banks: WP halves live at free offsets 0 (512 wide) and 512 (356)
        ps = ctx.enter_context(nc.psum_tensor("ps", [128, 1024], F32))
        psq = ctx.enter_context(nc.psum_tensor("psq", [128, 512], F32))
        psd = ctx.enter_context(nc.psum_tensor("psd", [128, 512], F32))

        s_sy = ctx.enter_context(nc.semaphore(name="s_sy"))     # ckq
        s_pw = ctx.enter_context(nc.semaphore(name="s_pw"))
        s_gp = ctx.enter_context(nc.semaphore(name="s_gp"))
        s_w = [ctx.enter_context(nc.semaphore(name=f"s_w{c}")) for c in range(N_WCHUNK)]
        s_pe = ctx.enter_context(nc.semaphore(name="s_pe"))
        s_dve = ctx.enter_context(nc.semaphore(name="s_dve"))
        s_act = ctx.enter_context(nc.semaphore(name="s_act"))

        block = ctx.enter_context(nc.Block(no_gpsimd_drain=True))

        # DMA plan: the whole w stream rides the gpsimd SWDGE ring (the only
        # ring that sustains ~280 GB/s) in chunks of [2,4,4,4,2] segments —
        # small first chunk so the PE starts early, small last chunk so only
        # ~0.7us of matmul remains after the final DMA lands.  The tiny fp8
        # pw/gp/ckq operands ride the SP/Activation HWDGE rings and arrive
        # during the SWDGE ramp-up.
        SWCH = [(0, 2), (2, 6), (6, 10), (10, 14), (14, 16)]

        @block.gpsimd
        def _(gpsimd):
            for c, (lo, hi) in enumerate(SWCH):
                gpsimd.dma_start(
                    wqt.ap()[:, lo * FREE:hi * FREE],
                    wq[:, lo * FREE:hi * FREE],
                ).then_inc(s_w[c], 16)

        @block.sync
        def _(sync):
            sync.dma_start(pwt.ap(), pw).then_inc(s_pw, 16)
            sync.dma_start(ckqt.ap(), ckq).then_inc(s_sy, 16)
            sync.wait_ge(s_act, 3)
            sync.dma_start(acc_out, acc.ap()).then_inc(s_sy, 16)

        @block.tensor
        def _(tensor):
            # warm up the PE p-state ramp on throwaway matmuls while w lands
            tensor.wait_ge(s_pw, 16)
            for _i in range(3):
                tensor.matmul(
                    psd.ap()[0:32, 0:HA],
                    pwt.ap()[:NPART, 0:32],
                    pwt.ap()[:NPART, 0:HA],
                    start=True, stop=True,
                )
            # quad: only needs ckq, runs while w still streams in
            tensor.wait_ge(s_sy, 16)
            tensor.matmul(
                psq.ap()[:8, 0:8],
                ckqt.ap()[:, 0:8],
                ckqt.ap()[:, 0:8],
                start=True, stop=True,
            ).then_inc(s_pe, 1)
            # 16 segments in 4 PSUM quadrants; within a quadrant run all four
            # h=0 halves then all four h=1 halves so the h=0 accumulation
            # groups close a few matmuls before the very end (the DVE h=0
            # multiply overlaps the tail h=1 matmuls of the last quadrant).
            # chunk needed before (g, j) in the h=0 pass: seg s = 4g+j maps
            # to chunk 0:{0,1} 1:{2..5} 2:{6..9} 3:{10..13} 4:{14,15}; wait
            # right before the first matmul that consumes each chunk.
            CHUNK_WAIT = {(0, 0): 0, (0, 2): 1, (1, 2): 2, (2, 2): 3, (3, 2): 4}
            for g in range(4):
                for h in range(2):
                    for j in range(4):
                        s = 4 * g + j
                        if h == 0 and (g, j) in ((0, 2), (1, 2)):
                            # keep the PE p-state ramp alive across the two
                            # chunk waits that usually find the PE idle
                            for _i in range(2 if g == 0 else 1):
                                tensor.matmul(
                                    psd.ap()[0:32, 0:HA],
                                    pwt.ap()[:NPART, 0:32],
                                    pwt.ap()[:NPART, 0:HA],
                                    start=True, stop=True,
                                )
                        if h == 0 and (g, j) in CHUNK_WAIT:
                            tensor.wait_ge(s_w[CHUNK_WAIT[(g, j)]], 16)
                        lo, hi = (0, HA) if h == 0 else (HA, FREE)
                        mm = tensor.matmul(
                            ps.ap()[32 * g:32 * g + 32, lo:hi],
                            pwt.ap()[:NPART, 32 * s:32 * s + 32],
                            wqt.ap()[:NPART, s * FREE + lo:s * FREE + hi],
                            start=(j == 0), stop=(j == 3),
                            tile_position=(0, 32 * g),
                        )
                        if g == 3 and j == 3:
                            mm.then_inc(s_pe, 1)

        @block.vector
        def _(vector):
            vector.memset(acc.ap(), 0.0)
            # tiny quad product: psq[:8,:8] * q8
            vector.wait_ge(s_pe, 1)
            vector.tensor_mul(
                out=scrapq.ap(),
                in0=psq.ap()[:8, 0:8],
                in1=ckqt.ap()[:8, 8:16].bitcast(F32),
            ).then_inc(s_dve, 1)
            # WP * G' in two free-halves so the ACT reduce pipelines behind
            vector.wait_ge(s_gp, 16)
            vector.wait_ge(s_pe, 2)
            vector.tensor_mul(
                out=scrap.ap()[:, 0:HA],
                in0=ps.ap()[:, 0:HA],
                in1=gpt.ap()[:, 0:HA],
            ).then_inc(s_dve, 1)
            vector.wait_ge(s_pe, 3)
            vector.tensor_mul(
                out=scrap.ap()[:, HA:FREE],
                in0=ps.ap()[:, HA:FREE],
                in1=gpt.ap()[:, HA:FREE],
            ).then_inc(s_dve, 1)

        @block.scalar
        def _(scalar):
            scalar.dma_start(gpt.ap(), gp).then_inc(s_gp, 16)
            scalar.wait_ge(s_dve, 1)
            scalar.activation(
                out=scrapq2.ap(), in_=scrapq.ap(),
                func=mybir.ActivationFunctionType.Copy,
                accum_out=acc.ap()[:8, 1:2],
            ).then_inc(s_act, 1)
            scalar.wait_ge(s_dve, 2)
            scalar.activation(
                out=scrap2.ap()[:, 0:HA], in_=scrap.ap()[:, 0:HA],
                func=mybir.ActivationFunctionType.Copy,
                accum_out=acc.ap()[:, 0:1],
            ).then_inc(s_act, 1)
            scalar.wait_ge(s_dve, 3)
            scalar.activation(
                out=scrap2.ap()[:, HA:FREE], in_=scrap.ap()[:, HA:FREE],
                func=mybir.ActivationFunctionType.Copy,
                accum_out=acc.ap()[:, 2:3],
            ).then_inc(s_act, 1)

    return nc


def _precompute(coeff, cost_mat, ts, w, num_steps):
    """Host-side prep of the tiny per-core operands + padded w blocks."""
    N = int(num_steps)
    ts = np.asarray(ts, np.float32)
    coeff = np.asarray(coeff, np.float32)
    w = np.asarray(w, np.float32)

    times = np.linspace(np.float32(ts[0]), np.float32(ts[-1]), N, dtype=np.float32)
    k = np.searchsorted(ts[1:-1], times, side="left")
    counts = np.bincount(k, minlength=NUM_SEG)
    starts = np.concatenate([[0], np.cumsum(counts)[:-1]]).astype(np.int64)
    assert counts.max() <= NPART * M_STEPS

    # G[seg, s, e] : per-output-row polynomial coefficients in dt^e
    d_of_s = np.array([0, 0, 0, 1, 1, 1, 2, 2, 2, 3, 3, 3, 0, 1])
    a_of_s = np.array([0, 1, 2, 0, 1, 2, 0, 1, 2, 0, 1, 2, 3, 3])
    G = np.zeros((NUM_SEG, 14, NC8), np.float64)
    for s in range(14):
        d, a = int(d_of_s[s]), int(a_of_s[s])
        for e in range(NC8 - d):
            G[:, s, e] = _falling(e + d, d) * coeff[a, :, e + d].astype(np.float64)

    # T[q, e, e'] = C(e,e') (q h)^(e-e')
    from math import comb
    h = (np.float64(ts[-1]) - np.float64(ts[0])) / (N - 1)
    T = np.zeros((M_STEPS, NC8, NC8), np.float64)
    for q in range(M_STEPS):
        for e in range(NC8):
            for ep in range(e + 1):
                T[q, e, ep] = comb(e, ep) * (q * h) ** (e - ep)
    Gp = np.einsum("qef,kse->kqsf", T, G)              # (128, 62, 14, 8)
    rhs_all = np.ascontiguousarray(
        Gp.transpose(0, 3, 1, 2).reshape(NUM_SEG, NC8, FREE)).astype(np.float32)

    # per-partition base dt powers (zeros for inactive partitions)
    u = np.arange(NPART)
    n_act = -(-counts // M_STEPS)                      # ceil
    idx = np.minimum(starts[:, None] + M_STEPS * u[None, :], N - 1)
    dtb = times[idx].astype(np.float64) - ts.astype(np.float64)[:NUM_SEG, None]
    mask = u[None, :] < n_act[:, None]
    dtb = dtb * mask
    pows = dtb[:, None, :] ** np.arange(NC8)[None, :, None]   # (128, 8, 128)
    pows = pows * mask[:, None, :]                            # [seg, e, u]

    # padded per-segment w blocks, scaled and quantized to fp8 e4m3
    f8np = mybir.dt.np(F8)
    w_scaled = (w[14:].astype(np.float32) * np.float32(W_SCALE)).astype(f8np)
    wb_all = np.zeros((NUM_SEG, NPART * FREE), f8np)
    for seg in range(NUM_SEG):
        st, cnt = int(starts[seg]), int(counts[seg])
        wb_all[seg, : 14 * cnt] = w_scaled[14 * st: 14 * (st + cnt)]
    wb_all = wb_all.reshape(NUM_SEG, NPART, FREE)

    cost_mat = np.asarray(cost_mat, np.float32)
    q8b = np.ascontiguousarray(cost_mat[:NC8, :NC8])

    bf16 = mybir.dt.np(BF16)
    in_maps = []
    for c in range(N_CORES):
        sl = slice(c * SEG_PER_CORE, (c + 1) * SEG_PER_CORE)
        wbc = wb_all[sl]                                  # (16, 128, 868)
        wbc = wbc.transpose(1, 0, 2).reshape(NPART, SEG_PER_CORE * FREE)

        # zero-padded 32-wide stationaries: pw[u, 32 s + 8 (s%4) + e] = pow
        pw = np.zeros((128, SEG_PER_CORE * 32), np.float32)
        pc = pows[sl]                                     # (16, 8, 128)
        for s in range(SEG_PER_CORE):
            base = 32 * s + 8 * (s % 4)
            pw[:NPART, base:base + 8] = pc[s].T

        # G' stack matching the packed PSUM layout: gp[8 s + e, :] = G'[seg s, e]
        gpc = rhs_all[sl].reshape(SEG_PER_CORE * NC8, FREE)

        ckq = np.zeros((64, 16), np.float32)
        ckq[:, 0:8] = coeff[:4, sl, :].reshape(4 * SEG_PER_CORE, NC8)
        ckq[0:8, 8:16] = q8b

        f8lim = np.float32(448.0)
        pw8 = np.clip(pw, -f8lim, f8lim).astype(f8np)
        gp8 = np.clip(gpc * np.float32(GP_SCALE), -f8lim, f8lim).astype(f8np)
        in_maps.append({
            "wq": np.ascontiguousarray(wbc),
            "pw": np.ascontiguousarray(pw8),
            "gp": np.ascontiguousarray(gp8),
            "ckq": np.ascontiguousarray(ckq),
        })
    return in_maps


def _install_ntff_hook_shim():
    """The agent image lacks ``antenv.axon_hooks``; recreate it so
    run_bass_kernel_spmd's trace=True path can find the NTFF profile hook
    (test-only; the grading path never passes _trace)."""
    import sys, types
    if "antenv.axon_hooks" in sys.modules:
        return
    import antenv
    mod = types.ModuleType("antenv.axon_hooks")
    _h = [None]
    mod.set_axon_ntff_profile_hook = lambda h: _h.__setitem__(0, h)
    mod.get_axon_ntff_profile_hook = lambda: _h[0]
    sys.modules["antenv.axon_hooks"] = mod
    antenv.axon_hooks = mod
    try:
        from trn_agent_boot.trn_boot import _ntff_profile_via_ctypes
        mod.set_axon_ntff_profile_hook(
            _ntff_profile_via_ctypes("/opt/axon/libaxon_pjrt.so"))
    except Exception as e:
        print("ntff hook shim failed:", e)


def kernel(coeff, cost_mat, ts, x0, w_reg, rho, p, num_steps,
           _trace=False, _trace_cores=None):
    global LAST_RESULTS
    coeff = np.asarray(coeff)
    cost_mat = np.asarray(cost_mat)
    ts = np.asarray(ts)
    x0 = np.asarray(x0)
    w_reg = np.asarray(w_reg)
    assert int(p) == 4 and int(num_steps) == 1_000_000

    cost_mat32 = np.asarray(cost_mat, np.float32)
    q8b = cost_mat32[:NC8, :NC8]
    kron_ok = np.array_equal(
        cost_mat32, np.kron(np.eye(NUM_SEG, dtype=np.float32), q8b))
    in_maps = _precompute(coeff, cost_mat, ts, w_reg, num_steps)
    nc = _build_nc()
    kwargs = {}
    if _trace:
        _install_ntff_hook_shim()
        kwargs = dict(trace=True, trace_cores=_trace_cores or [0])
    res = run_bass_kernel_spmd(nc, in_maps, list(range(N_CORES)), **kwargs)
    LAST_RESULTS = res

    quad = 0.0
    reg = 0.0
    for c in range(N_CORES):
        acc = np.asarray(res.results[c]["acc_out"], np.float64)
        reg += (acc[:, 0] + acc[:, 2]).sum() / (W_SCALE * GP_SCALE)
        quad += acc[:8, 1].sum()
    reg += float(np.asarray(w_reg[:14], np.float64) @ np.asarray(x0, np.float64))
    if not kron_ok:
        # cost_mat without the expected kron structure: the on-device quad
        # fast path does not apply; recompute the (tiny) quadratic exactly.
        C = np.asarray(coeff, np.float64)[:4].reshape(4, -1)
        quad = float(np.einsum("pi,ij,pj->", C, np.asarray(cost_mat, np.float64), C))
    return np.float32(quad + float(rho) * reg)
